# revision 6
# baseline (speedup 1.0000x reference)
"""Cross-attention Trainium2 kernel (B=8, N=2048, C=768, head=1).

reference:
  q = q_x @ Wq.T ; k = k_x @ Wk.T
  S = (q @ k.T) / 768 ; P = softmax(S, -1) ; out = P @ v_x

Strategy (per core, data-parallel over batch):
  M  = Wq.T @ Wk                 (768x768, both operands in direct layout)
  tT = (q_x @ M).T   [c2, n]     (q_x.T via PE transpose)
  ST[m, n] = sum_c2 k_x[m,c2] * tT[c2,n]   lhsT = k_x.T (PE transpose)
  PT = exp(ST / 768) [m, n]      (no max-subtraction: |S/768| < ~0.3)
  O[n, 0:770] = PT.T @ [v_x | 1 | 1] -> col 768 is the softmax denominator
  (two ones columns: fp32r matmul dst free-count must be even)
  out[n, c] = O[n, c] / O[n, 768]

Matmuls run as float32r (fp32-range, ~12-bit mantissa, full PE rate at
free>=256; ~2.4e-4 max input rounding error). Every matmul operand is
produced by an on-chip copy/activation that performs the fp32->fp32r
rounding walrus requires.
"""

import sys

sys.path.insert(0, "/opt/trn_rl_repo")

from contextlib import ExitStack

import numpy as np

import concourse.bass as bass
import concourse.mybir as mybir
import concourse.tile as tile
from concourse import bacc
from concourse.masks import make_identity

F32 = mybir.dt.float32
F32R = mybir.dt.float32r

B = 8
N = 2048
C = 768
P = 128
CC = C // P          # 6 chunks of the channel dim
NN = N // P          # 16 chunks of the sequence dim
BLK = 512            # free-dim block (PSUM bank = 512 f32)
NB = N // BLK        # 4 sequence blocks
SCALE = 1.0 / float(C)
EXP = mybir.ActivationFunctionType.Exp
COPY = mybir.ActivationFunctionType.Copy


def build_kernel():
    nc = bacc.Bacc("TRN2", target_bir_lowering=False, debug=False, num_devices=B)
    q_x = nc.declare_dram_parameter("q_x", [N, C], F32, isOutput=False)
    k_x = nc.declare_dram_parameter("k_x", [N, C], F32, isOutput=False)
    v_x = nc.declare_dram_parameter("v_x", [N, C], F32, isOutput=False)
    Wq = nc.declare_dram_parameter("Wq", [C, C], F32, isOutput=False)
    Wk = nc.declare_dram_parameter("Wk", [C, C], F32, isOutput=False)
    out = nc.declare_dram_parameter("out", [N, C], F32, isOutput=True)

    with tile.TileContext(nc) as tc, ExitStack() as ctx:
        persist = ctx.enter_context(tc.tile_pool(name="persist", bufs=1))
        tT = persist.tile([P, CC, N], F32R)      # (q_x @ M).T : [c2, n]
        kT = persist.tile([P, CC, N], F32R)      # k_x.T       : [c2, m]
        ident = persist.tile([P, P], F32)
        make_identity(nc, ident)

        # v' = [v_x | ones] : [m, 769] in fp32r (staged + rounded below)
        vpool = ctx.enter_context(tc.tile_pool(name="vpool", bufs=1))
        vb = vpool.tile([P, NN, C + 2], F32R)
        ones = persist.tile([P, NN, 2], F32)
        nc.vector.memset(ones, 1.0)
        nc.vector.tensor_copy(out=vb[:, :, C : C + 2], in_=ones)

        # ---------------- phase A: M, kT, tT ----------------
        with (
            tc.tile_pool(name="m_pool", bufs=1) as m_pool,
            tc.tile_pool(name="stage", bufs=4) as stage,
            tc.tile_pool(name="stage_r", bufs=2) as stage_r,
            tc.tile_pool(name="stage_s", bufs=2) as stage_s,
            tc.tile_pool(name="qxt", bufs=1) as qxt_pool,
        ):
            sbM = m_pool.tile([P, CC, C], F32R)  # M[c1, c2]

            # v: stage fp32 chunks, round into vb (overlaps M below via Tile)
            for mc in range(NN):
                v_t = stage.tile([P, C], F32, tag="ld", name=f"v{mc}")
                nc.sync.dma_start(out=v_t, in_=v_x[mc * P : (mc + 1) * P, :])
                nc.vector.tensor_copy(out=vb[:, mc, 0:C], in_=v_t)

            # --- M = Wq.T @ Wk, c2 in halves of 384 ---
            with tc.tile_pool(name="m_psum", bufs=6, space="PSUM") as m_psum:
                for h in range(2):
                    psums = [
                        m_psum.tile([P, 384], F32, tag="mp", name=f"mp{h}_{i}")
                        for i in range(CC)
                    ]
                    for dc in range(CC):
                        wq_d = stage.tile([P, C], F32, tag="ld", name=f"wq{h}_{dc}")
                        nc.sync.dma_start(out=wq_d, in_=Wq[dc * P : (dc + 1) * P, :])
                        wq_r = stage_r.tile([P, C], F32R, tag="ldr", name=f"wqr{h}_{dc}")
                        nc.vector.tensor_copy(out=wq_r, in_=wq_d)
                        wk_d = stage_s.tile([P, 384], F32, tag="lds", name=f"wk{h}_{dc}")
                        nc.sync.dma_start(
                            out=wk_d,
                            in_=Wk[dc * P : (dc + 1) * P, h * 384 : (h + 1) * 384],
                        )
                        wk_r = stage_s.tile([P, 384], F32R, tag="ldsr", name=f"wkr{h}_{dc}")
                        nc.vector.tensor_copy(out=wk_r, in_=wk_d)
                        for c1c in range(CC):
                            nc.tensor.matmul(
                                psums[c1c],
                                wq_r[:, c1c * P : (c1c + 1) * P],
                                wk_r,
                                start=(dc == 0),
                                stop=(dc == CC - 1),
                            )
                    for c1c in range(CC):
                        nc.vector.tensor_copy(
                            out=sbM[:, c1c, h * 384 : (h + 1) * 384], in_=psums[c1c]
                        )

            with (
                tc.tile_pool(name="tr_psum", bufs=2, space="PSUM") as tr_psum,
                tc.tile_pool(name="tt_psum", bufs=2, space="PSUM") as tt_psum,
            ):
                # --- kT: transpose k_x in groups of 4 n-chunks ---
                for g in range(NN // 4):
                    tiles = []
                    for j in range(4):
                        kx_t = stage.tile([P, C], F32, tag="ld", name=f"kx{g}_{j}")
                        nc.sync.dma_start(
                            out=kx_t, in_=k_x[(4 * g + j) * P : (4 * g + j + 1) * P, :]
                        )
                        tiles.append(kx_t)
                    for cc in range(CC):
                        ps = tr_psum.tile([P, BLK], F32, tag="trp", name=f"kps{g}_{cc}")
                        for j in range(4):
                            nc.tensor.transpose(
                                ps[:, j * P : (j + 1) * P],
                                tiles[j][:, cc * P : (cc + 1) * P],
                                ident,
                            )
                        nc.vector.tensor_copy(
                            out=kT[:, cc, g * BLK : (g + 1) * BLK], in_=ps
                        )

                # --- per n-block: transpose q_x block, then tT block ---
                for nb in range(NB):
                    qxT = qxt_pool.tile([P, CC, BLK], F32R, tag="qxT", name=f"qxT{nb}")
                    tiles = []
                    for j in range(4):
                        qx_t = stage.tile([P, C], F32, tag="ld", name=f"qx{nb}_{j}")
                        nc.sync.dma_start(
                            out=qx_t, in_=q_x[(4 * nb + j) * P : (4 * nb + j + 1) * P, :]
                        )
                        tiles.append(qx_t)
                    for cc in range(CC):
                        ps = tr_psum.tile([P, BLK], F32, tag="trp", name=f"qps{nb}_{cc}")
                        for j in range(4):
                            nc.tensor.transpose(
                                ps[:, j * P : (j + 1) * P],
                                tiles[j][:, cc * P : (cc + 1) * P],
                                ident,
                            )
                        nc.vector.tensor_copy(out=qxT[:, cc, :], in_=ps)
                    for c2c in range(CC):
                        tps = tt_psum.tile([P, BLK], F32, tag="ttp", name=f"tps{nb}_{c2c}")
                        for c1c in range(CC):
                            nc.tensor.matmul(
                                tps,
                                sbM[:, c1c, c2c * P : (c2c + 1) * P],
                                qxT[:, c1c, :],
                                start=(c1c == 0),
                                stop=(c1c == CC - 1),
                            )
                        nc.vector.tensor_copy(
                            out=tT[:, c2c, nb * BLK : (nb + 1) * BLK], in_=tps
                        )

        # ---------------- phase B: attention ----------------
        with (
            tc.tile_pool(name="pt_pool", bufs=1) as pt_pool,
            tc.tile_pool(name="out_pool", bufs=2) as out_pool,
            tc.tile_pool(name="rec_pool", bufs=2) as rec_pool,
            tc.tile_pool(name="s_psum", bufs=3, space="PSUM") as s_psum,
            tc.tile_pool(name="o_psum", bufs=2, space="PSUM") as o_psum,
            tc.tile_pool(name="o2_psum", bufs=2, space="PSUM") as o2_psum,
        ):
            PT = pt_pool.tile([P, NN, BLK], F32R)
            for nb in range(NB):
                # ST[:, nb-block] per m-chunk, then exp -> PT (ACT rounds to f32r)
                for mc in range(NN):
                    sp = s_psum.tile([P, BLK], F32, tag="sp", name=f"sp{nb}_{mc}")
                    for c2c in range(CC):
                        nc.tensor.matmul(
                            sp,
                            kT[:, c2c, mc * P : (mc + 1) * P],
                            tT[:, c2c, nb * BLK : (nb + 1) * BLK],
                            start=(c2c == 0),
                            stop=(c2c == CC - 1),
                        )
                    nc.scalar.activation(
                        out=PT[:, mc, :], in_=sp, func=EXP, scale=SCALE
                    )
                # PV: O[n_sub, 769] = PT.T @ v'
                for ns in range(4):
                    op1 = o_psum.tile([P, BLK], F32, tag="op1", name=f"o1_{nb}_{ns}")
                    op2 = o2_psum.tile(
                        [P, C + 2 - BLK], F32, tag="op2", name=f"o2_{nb}_{ns}"
                    )
                    for mc in range(NN):
                        lhs = PT[:, mc, ns * P : (ns + 1) * P]
                        nc.tensor.matmul(
                            op1, lhs, vb[:, mc, 0:BLK],
                            start=(mc == 0), stop=(mc == NN - 1),
                        )
                        nc.tensor.matmul(
                            op2, lhs, vb[:, mc, BLK : C + 2],
                            start=(mc == 0), stop=(mc == NN - 1),
                        )
                    rec = rec_pool.tile([P, 1], F32, tag="rec", name=f"rc{nb}_{ns}")
                    nc.vector.reciprocal(out=rec, in_=op2[:, C - BLK : C - BLK + 1])
                    o_t = out_pool.tile([P, C], F32, tag="ot", name=f"ot{nb}_{ns}")
                    nc.scalar.activation(
                        out=o_t[:, 0:BLK], in_=op1, func=COPY, scale=rec
                    )
                    nc.scalar.activation(
                        out=o_t[:, BLK:C], in_=op2[:, 0 : C - BLK], func=COPY, scale=rec
                    )
                    row0 = nb * BLK + ns * P
                    nc.sync.dma_start(out=out[row0 : row0 + P, :], in_=o_t)

    nc.compile()
    return nc


_NC = None


def _get_nc():
    global _NC
    if _NC is None:
        _NC = build_kernel()
    return _NC


def kernel(q_x, k_x, v_x, Wq, Wk):
    from concourse.bass_utils import run_bass_kernel_spmd

    q_x = np.ascontiguousarray(np.asarray(q_x, dtype=np.float32))
    k_x = np.ascontiguousarray(np.asarray(k_x, dtype=np.float32))
    v_x = np.ascontiguousarray(np.asarray(v_x, dtype=np.float32))
    Wq = np.ascontiguousarray(np.asarray(Wq, dtype=np.float32))
    Wk = np.ascontiguousarray(np.asarray(Wk, dtype=np.float32))

    nc = _get_nc()
    in_maps = [
        {"q_x": q_x[i], "k_x": k_x[i], "v_x": v_x[i], "Wq": Wq, "Wk": Wk}
        for i in range(B)
    ]
    res = run_bass_kernel_spmd(nc, in_maps, core_ids=list(range(B)))
    return np.stack([res.results[i]["out"] for i in range(B)], axis=0)
